# revision 24
# baseline (speedup 1.0000x reference)
"""Causal single-head attention on 8 Trainium2 NeuronCores.

Problem: x[8, 2048, 1024] -> out[8, 2048, 64]
  q/k/v = x @ W{q,k,v} + b{q,k,v};  out = softmax(causal(q k^T / 8)) v

Sharding: data-parallel over batch; core b computes batch element b.

Per-core design (T=2048, D=1024, H=64), all matmul operands bf16 with
fp32 PSUM accumulation; transposes and the output path in f32r/f32:
  - host sends x[b]^T as xt [D, T] (bf16) so D sits on partitions
  - QKV: lhsT=[Wq|Wk] chunk [128d, 128] (full PE array), rhs=xt chunk
    [128d, 512t] -> psum [128(qh|kh), 512]; bias added during the
    PSUM->SBUF copy, giving qT/kT [64h, T].  V: lhsT=Wv (M=64), then
    PE-transposed to natural v tiles [128t, 64h] + ones/zeros columns
    -> v_sb [128, 16, 66].
  - attention in four i-quarters of 512: for each quarter iq and each
    causal j-chunk jt (128 rows): S^T psum [128j, <=512i] = kT_c.T@qT;
    exp via ACT (scale=1/8 fused) psum->P (bf16); diagonal 128-block
    causally masked by affine_select; PV accumulates
    psum_out[66, 512] += [v_jt|1|0].T @ P, whose row 64 is the softmax
    denominator for free.
  - out: PE-transpose [66,128] blocks -> [128, 66], divide rows by the
    denominator column (per-partition scalar after transpose), DMA out.
  - all PSUM tiles are one bank (512 f32) -> 6 rotating work slots + 2
    output-accumulator slots; PE warmup matmuls + an early dummy exp
    keep the HAM clock gate at 8/8 and preload the ACT table.
"""

import os
from contextlib import ExitStack

import ml_dtypes
import numpy as np

import concourse.bacc as bacc
import concourse.mybir as mybir
import concourse.tile as tile
from concourse.bass_utils import run_bass_kernel_spmd

F32 = mybir.dt.float32
F32R = mybir.dt.float32r
BF16 = mybir.dt.bfloat16
AF = mybir.ActivationFunctionType
ALU = mybir.AluOpType

T = 2048
D = 1024
H = 64
NB = 8
DC = D // 128      # 8 contraction chunks
NJT = T // 128     # 16 j-chunks (also 16 t-tiles)
QW = 512           # i-quarter width
NQ = T // QW       # 4 quarters
SCALE = 1.0 / 8.0  # 1/sqrt(H)

_CACHE: dict = {}


def _emit_v_group(nc, lo, xt_sb, wv_sb, bv_sb, vT, ps):
    sl = slice(lo, lo + 512)
    ps_v = ps.tile([64, 512], F32, tag="s", name=f"psv{lo}")
    for c in range(DC):
        nc.tensor.matmul(
            ps_v[:], wv_sb[:, c, :], xt_sb[:, c, sl],
            start=(c == 0), stop=(c == DC - 1),
        )
    nc.vector.tensor_scalar(
        out=vT[:, sl], in0=ps_v[:],
        scalar1=bv_sb[:], scalar2=None, op0=ALU.add,
    )


def _emit_v_transpose(nc, lo, vT, v_sb, ident, ps):
    # transpose v^T [64, 128]-tiles -> natural v [128, 64] tiles
    ps_t = ps.tile([128, 4, H], F32, tag="s", name=f"pst{lo}")
    for j2 in range(4):
        jt = lo // 128 + j2
        nc.tensor.transpose(
            ps_t[:, j2, :].bitcast(F32R),
            vT[:, jt * 128:(jt + 1) * 128],
            ident[0:64, 0:64],
        )
    nc.vector.tensor_copy(
        v_sb[:, lo // 128:lo // 128 + 4, 0:H], ps_t[:, :, :])


def _emit_qkv_half(nc, th, xt_sb, wqk_sb, wv_sb, bqk_sb, bv_sb,
                   qT, kT, vT, v_sb, ident, ps, defer_v=False):
    """QKV for t in [th*1024, (th+1)*1024); optionally defer the V pass."""
    v_closures = []
    for t2 in range(2):
        lo = th * 1024 + t2 * 512
        sl = slice(lo, lo + 512)
        ps_qk = ps.tile([128, 512], F32, tag="s")
        for c in range(DC):
            nc.tensor.matmul(
                ps_qk[:], wqk_sb[:, c, :], xt_sb[:, c, sl],
                start=(c == 0), stop=(c == DC - 1),
            )
        nc.vector.tensor_scalar(
            out=qT[:, sl], in0=ps_qk[0:64, :],
            scalar1=bqk_sb[0:64, :], scalar2=None, op0=ALU.add,
        )
        nc.vector.tensor_scalar(
            out=kT[:, sl], in0=ps_qk[64:128, :],
            scalar1=bqk_sb[64:128, :], scalar2=None, op0=ALU.add,
        )
        if defer_v:
            v_closures.append(
                lambda lo=lo: _emit_v_group(nc, lo, xt_sb, wv_sb, bv_sb,
                                            vT, ps))
            v_closures.append(
                lambda lo=lo: _emit_v_transpose(nc, lo, vT, v_sb, ident, ps))
        else:
            _emit_v_group(nc, lo, xt_sb, wv_sb, bv_sb, vT, ps)
            _emit_v_transpose(nc, lo, vT, v_sb, ident, ps)
    return v_closures


def _emit_attn_pair(nc, iqa, iqb, qT, kT, v_sb, ps, out_ps, ppool,
                    fillers=None):
    """One i-halfwindow [iqa*QW, (iqb+1)*QW) per 1024-wide strip: one exp
    per j-chunk (halves ACT fixed cost), PV split into the two quarter
    accumulators.

    `fillers`: optional list of zero-arg closures (deferred real work);
    one fires after each jt round to fill PE exp-wait slivers and keep
    the HAM clock at 8/8.
    """
    assert iqb == iqa + 1
    w0 = iqa * QW                       # window start in i
    jt_max = min(NJT, 4 * (iqb + 1))    # causal j-chunks for the window
    jma = min(NJT, 4 * (iqa + 1))       # last writer of quarter a is jma-1
    ps_oa = out_ps.tile([66, QW], F32, tag="out")
    ps_ob = out_ps.tile([66, QW], F32, tag="out")
    for jt in range(jt_max):
        off = max(128 * jt - w0, 0)     # within [0, 1024)
        ps_s = ps.tile([128, 2 * QW], F32, tag="w")
        for h in range(2):
            a, b = max(off, h * QW), (h + 1) * QW
            if a < b:
                nc.tensor.matmul(
                    ps_s[:, a:b],
                    kT[:, jt * 128:(jt + 1) * 128],
                    qT[:, w0 + a: w0 + b],
                    start=True, stop=True,
                )
        P = ppool.tile([128, 2 * QW], BF16, tag="P")
        nc.scalar.activation(
            out=P[:, off:], in_=ps_s[:, off:], func=AF.Exp, scale=SCALE,
        )
        if 128 * jt >= w0:
            # diagonal block: keep i >= j  (i = w0+off+f, j = 128*jt+p)
            nc.gpsimd.affine_select(
                out=P[:, off:off + 128], in_=P[:, off:off + 128],
                compare_op=ALU.is_ge, fill=0.0,
                base=0, pattern=[[1, 128]], channel_multiplier=-1,
            )
        if off < QW and jt < jma:
            nc.tensor.matmul(
                ps_oa[:, off:QW], v_sb[:, jt, :], P[:, off:QW],
                start=(jt == 0), stop=(jt == jma - 1),
            )
        offb = max(off - QW, 0)
        nc.tensor.matmul(
            ps_ob[:, offb:QW], v_sb[:, jt, :], P[:, QW + offb:],
            start=(jt == 0), stop=(jt == jt_max - 1),
        )
        if fillers:
            fillers.pop(0)()
    return ps_oa, ps_ob


def _emit_drain_copy(nc, otpool, ps_o):
    oT = otpool.tile([66, QW], F32R, tag="oT")
    nc.vector.tensor_copy(oT[:], ps_o[:])
    return oT


def _drain_closures(nc, iq, oT, ident, ps, out_nat, recip, out):
    """Drain work as closures: 4 transposes + 1 finish step."""
    state = {}

    def _tr(t2):
        def go():
            if "ps_n" not in state:
                state["ps_n"] = ps.tile([128, 4, 128], F32, tag="s", name=f"psn{iq}")
            nc.tensor.transpose(
                state["ps_n"][:, t2, 0:66].bitcast(F32R),
                oT[:, t2 * 128:(t2 + 1) * 128],
                ident[:, :],
            )
        return go

    def _fin():
        sl = slice(iq * 4, (iq + 1) * 4)
        nc.vector.tensor_copy(out_nat[:, sl, :], state["ps_n"][:, :, 0:66])
        nc.vector.reciprocal(recip[:, sl], out_nat[:, sl, H])
        for tt in range(iq * 4, (iq + 1) * 4):
            nc.vector.tensor_scalar_mul(
                out_nat[:, tt, 0:H], out_nat[:, tt, 0:H],
                recip[:, tt:tt + 1])
        nc.sync.dma_start(
            out=out.rearrange("(qq tt p) h -> qq p tt h", qq=NQ, p=128)[iq],
            in_=out_nat[:, sl, 0:H],
        )

    return [_tr(t) for t in range(4)] + [_fin]


def _emit_drain(nc, iq, oT, ident, ps, out_nat, recip, out):
    for go in _drain_closures(nc, iq, oT, ident, ps, out_nat, recip, out):
        go()


def _build():
    nc = bacc.Bacc("TRN2", target_bir_lowering=False, debug=False,
                   num_devices=NB)
    xt = nc.dram_tensor("xt", [D, T], BF16, kind="ExternalInput")
    wqk = nc.dram_tensor("wqk", [D, 128], BF16, kind="ExternalInput")
    wv = nc.dram_tensor("wv", [D, H], BF16, kind="ExternalInput")
    bqk = nc.dram_tensor("bqk", [128, 1], F32, kind="ExternalInput")
    bv = nc.dram_tensor("bv", [H, 1], F32, kind="ExternalInput")
    ident66 = nc.dram_tensor("ident66", [66, 66], F32R, kind="ExternalInput")
    vtail = nc.dram_tensor("vtail", [128, NJT, 2], BF16, kind="ExternalInput")
    out = nc.dram_tensor("out", [T, H], F32, kind="ExternalOutput")

    with ExitStack() as ctx:
        tc = ctx.enter_context(tile.TileContext(nc))
        const = ctx.enter_context(tc.tile_pool(name="const", bufs=1))
        big = ctx.enter_context(tc.tile_pool(name="big", bufs=1))
        ppool = ctx.enter_context(tc.tile_pool(name="ppool", bufs=4))
        otpool = ctx.enter_context(tc.tile_pool(name="otpool", bufs=2))
        ps = ctx.enter_context(tc.tile_pool(name="ps", bufs=2, space="PSUM"))
        pss = ctx.enter_context(tc.tile_pool(name="pss", bufs=2, space="PSUM"))
        out_ps = ctx.enter_context(
            tc.tile_pool(name="out_ps", bufs=2, space="PSUM"))

        # constants / weights
        wqk_sb = const.tile([128, DC, 128], BF16)
        nc.sync.dma_start(
            out=wqk_sb[:], in_=wqk.rearrange("(c p) m -> p c m", p=128))
        wv_sb = const.tile([128, DC, H], BF16)
        nc.sync.dma_start(
            out=wv_sb[:], in_=wv.rearrange("(c p) m -> p c m", p=128))
        bqk_sb = const.tile([128, 1], F32)
        nc.sync.dma_start(out=bqk_sb[:], in_=bqk[:])
        bv_sb = const.tile([H, 1], F32)
        nc.sync.dma_start(out=bv_sb[:], in_=bv[:])
        ident = const.tile([66, 66], F32R)
        nc.sync.dma_start(out=ident[:], in_=ident66[:])

        # x^T resident in SBUF (bf16), th-major halves for early compute
        xt_sb = big.tile([128, DC, T], BF16)
        for th in range(2):
            for c in range(DC):
                nc.sync.dma_start(
                    out=xt_sb[:, c, th * 1024:(th + 1) * 1024],
                    in_=xt[c * 128:(c + 1) * 128, th * 1024:(th + 1) * 1024],
                )

        qT = big.tile([64, T], BF16)
        kT = big.tile([64, T], BF16)
        vT = big.tile([64, T], F32R)
        v_sb = big.tile([128, NJT, H + 2], BF16)
        nc.sync.dma_start(out=v_sb[:, :, H:H + 2], in_=vtail[:])
        out_nat = big.tile([128, NJT, H + 2], F32)
        recip = const.tile([128, NJT], F32)

        # PE warmup + ACT table preload during the input-DMA window
        warm = const.tile([128, 512], BF16)
        nc.vector.memset(warm[:], 0.0)
        escr = const.tile([128, 2], F32)
        nc.vector.memset(escr[:], 0.0)
        nc.scalar.activation(
            out=escr[:], in_=escr[:], func=AF.Exp, scale=1.0,
        )
        ps_w = pss.tile([128, 512], F32, tag="s")
        for _ in range(24):
            nc.tensor.matmul(ps_w[:], warm[:, 0:128], warm[:],
                             start=True, stop=True)

        qkv = (qT, kT, vT, v_sb, ident, pss)
        wargs = (xt_sb, wqk_sb, wv_sb, bqk_sb, bv_sb)
        attn = (qT, kT, v_sb, ps, out_ps, ppool)
        drain = (ident, pss, out_nat, recip, out)

        vfill0 = _emit_qkv_half(nc, 0, *wargs, *qkv, defer_v=True)
        vfill0[0]()  # v group for jt 0-3: needed at round 0
        vfill0[1]()
        o0, o1 = _emit_attn_pair(nc, 0, 1, *attn, fillers=vfill0[2:])
        t0 = _emit_drain_copy(nc, otpool, o0)
        t1 = _emit_drain_copy(nc, otpool, o1)
        vfill = _emit_qkv_half(nc, 1, *wargs, *qkv, defer_v=True)
        fillers = (vfill
                   + _drain_closures(nc, 0, t0, *drain)
                   + _drain_closures(nc, 1, t1, *drain))
        o2, o3 = _emit_attn_pair(nc, 2, 3, *attn, fillers=fillers)
        t2 = _emit_drain_copy(nc, otpool, o2)
        t3 = _emit_drain_copy(nc, otpool, o3)
        _emit_drain(nc, 2, t2, *drain)
        _emit_drain(nc, 3, t3, *drain)

    nc.compile()
    return nc


def _get_nc():
    if "nc" not in _CACHE:
        _CACHE["nc"] = _build()
    return _CACHE["nc"]


def kernel(x, Wq, bq, Wk, bk, Wv, bv):
    x = np.ascontiguousarray(np.asarray(x, dtype=np.float32))
    Wq = np.asarray(Wq, dtype=np.float32)
    Wk = np.asarray(Wk, dtype=np.float32)
    Wv = np.ascontiguousarray(np.asarray(Wv, dtype=np.float32))
    bq = np.asarray(bq, dtype=np.float32)
    bk = np.asarray(bk, dtype=np.float32)
    bv = np.asarray(bv, dtype=np.float32)

    wqk = np.ascontiguousarray(
        np.concatenate([Wq, Wk], axis=1)).astype(ml_dtypes.bfloat16)
    wv_b = Wv.astype(ml_dtypes.bfloat16)
    x_b = x.astype(ml_dtypes.bfloat16)
    bqk = np.ascontiguousarray(np.concatenate([bq, bk])[:, None])
    bv_ = np.ascontiguousarray(bv[:, None])
    ident66 = np.eye(66, dtype=np.float32)
    vtail = np.zeros((128, NJT, 2), dtype=ml_dtypes.bfloat16)
    vtail[:, :, 0] = 1.0

    in_maps = []
    for b in range(NB):
        in_maps.append({
            "xt": np.ascontiguousarray(x_b[b].T),
            "wqk": wqk,
            "wv": wv_b,
            "bqk": bqk,
            "bv": bv_,
            "ident66": ident66,
            "vtail": vtail,
        })

    nc = _get_nc()
    trace = bool(int(os.environ.get("KTRACE", "0")))
    res = run_bass_kernel_spmd(
        nc, in_maps, core_ids=list(range(NB)), trace=trace,
    )
    if trace:
        _CACHE["exec_time_ns"] = res.exec_time_ns
        _CACHE["results"] = res
    return np.stack([r["out"] for r in res.results])
